# revision 1
# baseline (speedup 1.0000x reference)
"""Trainium2 kernel for nn_Atlas_154618823086 (fast-weight chunked TTT layer).

Sharding: tensor-parallel over heads. Core c of 8 owns heads [2c, 2c+1]
(= 128 of the 1024 channels). Two SPMD NEFFs:
  k1: y = hs @ [Wq|Wk|Wv|Wg].T[:, cols_c]   (16384x1024x512 per core, bf16)
  k2: partial_c = (o*gate)[:, cols_c] @ Wo.T[cols_c, :]  (row-parallel; host
      sums the 8 partials = the unshard step for partial-sum sharding)
The sequential 256-step fast-weight recurrence + short conv + norms run on
host between the two NEFFs (tiny FLOP count, latency-bound part).
"""
import numpy as np
import ml_dtypes
from contextlib import ExitStack

DIM = 1024
H = 16
HD = 64
DI = 4
CHUNK = 16
BASE_LR = 1e-3
KSZ = 4
B = 4
L = 4096
NCORES = 8
HPC = H // NCORES          # heads per core = 2
CPC = HPC * HD             # channels per core = 128
R = B * L                  # 16384 rows

bf16 = ml_dtypes.bfloat16

LAST_EXEC_NS = []


def _build_k1():
    import concourse.tile as tile
    import concourse.bass as bass
    from concourse import bacc, mybir

    nc = bacc.Bacc()
    f32 = mybir.dt.float32
    b16 = mybir.dt.bfloat16
    hsT = nc.dram_tensor("hsT", [DIM, R], b16, kind="ExternalInput")
    wT = nc.dram_tensor("wT", [DIM, 4 * CPC], b16, kind="ExternalInput")
    y = nc.dram_tensor("y", [R, 4 * CPC], b16, kind="ExternalOutput")

    NSTRIP = 512
    with tile.TileContext(nc) as tc, ExitStack() as ctx:
        wpool = ctx.enter_context(tc.tile_pool(name="w", bufs=1))
        xpool = ctx.enter_context(tc.tile_pool(name="x", bufs=3))
        opool = ctx.enter_context(tc.tile_pool(name="o", bufs=4))
        pspool = ctx.enter_context(
            tc.tile_pool(name="ps", bufs=4, space=bass.MemorySpace.PSUM))

        wt = wpool.tile([128, 8, 512], b16)
        for kt in range(8):
            nc.sync.dma_start(wt[:, kt, :], wT[kt * 128:(kt + 1) * 128, :])

        for s in range(R // NSTRIP):
            xt = xpool.tile([128, 8, NSTRIP], b16)
            for kt in range(8):
                nc.sync.dma_start(
                    xt[:, kt, :],
                    hsT[kt * 128:(kt + 1) * 128, s * NSTRIP:(s + 1) * NSTRIP])
            for m in range(NSTRIP // 128):
                ps = pspool.tile([128, 512], f32)
                for kt in range(8):
                    nc.tensor.matmul(ps[:], xt[:, kt, m * 128:(m + 1) * 128],
                                     wt[:, kt, :], start=(kt == 0),
                                     stop=(kt == 7))
                ot = opool.tile([128, 512], b16)
                nc.vector.tensor_copy(ot[:], ps[:])
                r0 = s * NSTRIP + m * 128
                nc.sync.dma_start(y[r0:r0 + 128, :], ot[:])
    nc.compile()
    return nc


def _build_k2():
    import concourse.tile as tile
    import concourse.bass as bass
    from concourse import bacc, mybir

    nc = bacc.Bacc()
    b16 = mybir.dt.bfloat16
    f32 = mybir.dt.float32
    ogT = nc.dram_tensor("ogT", [CPC, R], b16, kind="ExternalInput")
    woT = nc.dram_tensor("woT", [CPC, DIM], b16, kind="ExternalInput")
    par = nc.dram_tensor("par", [R, DIM], b16, kind="ExternalOutput")

    NSTRIP = 512
    with tile.TileContext(nc) as tc, ExitStack() as ctx:
        wpool = ctx.enter_context(tc.tile_pool(name="w", bufs=1))
        xpool = ctx.enter_context(tc.tile_pool(name="x", bufs=3))
        opool = ctx.enter_context(tc.tile_pool(name="o", bufs=4))
        pspool = ctx.enter_context(
            tc.tile_pool(name="ps", bufs=4, space=bass.MemorySpace.PSUM))

        wt = wpool.tile([128, DIM], b16)
        nc.sync.dma_start(wt[:], woT[:])

        for s in range(R // NSTRIP):
            xt = xpool.tile([128, NSTRIP], b16)
            nc.sync.dma_start(xt[:], ogT[:, s * NSTRIP:(s + 1) * NSTRIP])
            for m in range(NSTRIP // 128):
                ot = opool.tile([128, DIM], b16)
                for half in range(2):
                    ps = pspool.tile([128, 512], f32)
                    nc.tensor.matmul(ps[:], xt[:, m * 128:(m + 1) * 128],
                                     wt[:, half * 512:(half + 1) * 512],
                                     start=True, stop=True)
                    nc.vector.tensor_copy(ot[:, half * 512:(half + 1) * 512],
                                          ps[:])
                r0 = s * NSTRIP + m * 128
                nc.sync.dma_start(par[r0:r0 + 128, :], ot[:])
    nc.compile()
    return nc


_K1 = None
_K2 = None


def _run(nc, in_maps):
    import time
    from concourse.bass_utils import run_bass_kernel_spmd
    t0 = time.perf_counter()
    res = run_bass_kernel_spmd(nc, in_maps, core_ids=list(range(NCORES)))
    dt = time.perf_counter() - t0
    if res.exec_time_ns is not None:
        LAST_EXEC_NS.append(res.exec_time_ns)
    else:
        # no NTFF profiling in this container: wall-clock dispatch+exec proxy
        LAST_EXEC_NS.append(int(dt * 1e9))
    return res.results


def _softplus(x):
    return np.logaddexp(0.0, x)


def _silu(x):
    return x / (1.0 + np.exp(-x))


def _conv_residual(x, w):
    # x: (B, L, C) f32, w: (C, KSZ). causal depthwise conv + residual.
    y = 2.0 * x * 0.0  # zeros like x
    y += x * (1.0 + w[None, None, :, 3])  # j=3 tap aligns with t, plus residual
    for j in range(KSZ - 1):
        sh = KSZ - 1 - j  # 3,2,1
        y[:, sh:, :] += x[:, :-sh, :] * w[None, None, :, j]
    return y


def _attn(q, k, v):
    # q: (b, D, h, d), k/v: (b, T, h, d) -> (b, D, h, d); softmax over T
    s = np.einsum('bqhd,bkhd->bhqk', q, k) / np.sqrt(np.float32(q.shape[-1]))
    s -= s.max(-1, keepdims=True)
    p = np.exp(s)
    p /= p.sum(-1, keepdims=True)
    return np.einsum('bhqk,bkhd->bqhd', p, v)


def _softmax_last(x):
    x = x - x.max(-1, keepdims=True)
    e = np.exp(x)
    return e / e.sum(-1, keepdims=True)


def kernel(hidden_states, Wq, Wk, Wv, Wlr, Wg, Wo, cq, ck, cv,
           W_in_init, W_out_init, ln_g, ln_b):
    global _K1, _K2
    hs = np.asarray(hidden_states, np.float32)
    hsT = np.ascontiguousarray(
        hs.reshape(R, DIM).T).astype(bf16)  # (DIM, R)

    if _K1 is None:
        _K1 = _build_k1()
    in_maps = []
    for c in range(NCORES):
        cols = slice(CPC * c, CPC * (c + 1))
        wcat = np.concatenate(
            [np.asarray(W, np.float32).T[:, cols] for W in (Wq, Wk, Wv, Wg)],
            axis=1)  # (DIM, 512)
        in_maps.append({"hsT": hsT, "wT": np.ascontiguousarray(wcat).astype(bf16)})
    res1 = _run(_K1, in_maps)

    xq = np.empty((B, L, DIM), np.float32)
    xk = np.empty((B, L, DIM), np.float32)
    xv = np.empty((B, L, DIM), np.float32)
    gate = np.empty((B, L, DIM), np.float32)
    for c in range(NCORES):
        y = np.asarray(res1[c]["y"], np.float32).reshape(B, L, 4 * CPC)
        cols = slice(CPC * c, CPC * (c + 1))
        xq[:, :, cols] = y[:, :, 0 * CPC:1 * CPC]
        xk[:, :, cols] = y[:, :, 1 * CPC:2 * CPC]
        xv[:, :, cols] = y[:, :, 2 * CPC:3 * CPC]
        gate[:, :, cols] = y[:, :, 3 * CPC:4 * CPC]

    # host: conv + activations + norms + lr projection
    q = _silu(_conv_residual(xq, np.asarray(cq, np.float32)))
    k = _silu(_conv_residual(xk, np.asarray(ck, np.float32)))
    v = _silu(_conv_residual(xv, np.asarray(cv, np.float32)))
    q = q.reshape(B, L, H, HD)
    k = k.reshape(B, L, H, HD)
    v = v.reshape(B, L, H, HD)
    q = q / np.linalg.norm(q, axis=-1, keepdims=True)
    k = k / np.linalg.norm(k, axis=-1, keepdims=True)
    lr = _softplus(hs.reshape(R, DIM) @ np.asarray(Wlr, np.float32).T
                   + BASE_LR).reshape(B, L, H, 2)

    nchunk = L // CHUNK
    qc = q.reshape(B, nchunk, CHUNK, H, HD)
    kc = k.reshape(B, nchunk, CHUNK, H, HD)
    vc = v.reshape(B, nchunk, CHUNK, H, HD)
    lrc = lr.reshape(B, nchunk, CHUNK, H, 2)

    W_in = np.broadcast_to(np.asarray(W_in_init, np.float32),
                           (B, DI, H, HD)).copy()
    W_out = np.broadcast_to(np.asarray(W_out_init, np.float32),
                            (B, DI, H, HD)).copy()
    mask = np.tril(np.ones((CHUNK, CHUNK), np.float32))
    o = np.empty((B, nchunk, CHUNK, H, HD), np.float32)

    for t in range(nchunk):
        q_t = qc[:, t]
        k_t = kc[:, t]
        v_t = vc[:, t]
        lr_t = lrc[:, t]
        k_h = _softmax_last(np.einsum('blhd,bDhd->blhD', k_t, W_in)) \
            * lr_t[..., 1:]
        q_h = _softmax_last(np.einsum('blhd,bDhd->blhD', q_t, W_in))
        qk = np.einsum('bqhD,bkhD->bhqk', q_h, k_h) * mask[None, None]
        o[:, t] = (np.einsum('bqhD,bDhd->bqhd', q_h, W_out)
                   + np.einsum('bhqk,bkhd->bqhd', qk, v_t))
        W_out = W_out + np.einsum('bnhD,bnhd->bDhd', k_h, v_t)
        lr_in = lr_t[:, :1, :, 0:1]
        lr_out = lr_t[:, :1, :, 1:2]
        for _ in range(2):
            g_out = -_attn(W_in, k_t, v_t)
            g_in = -_attn(W_out, v_t, k_t)
            W_in = W_in - lr_in * g_in
            W_out = W_out - lr_out * g_out

    o = o.reshape(B, L, H, HD)
    mu = o.mean(-1, keepdims=True)
    var = ((o - mu) ** 2).mean(-1, keepdims=True)
    o = (o - mu) / np.sqrt(var + 1e-5) * np.asarray(ln_g, np.float32) \
        + np.asarray(ln_b, np.float32)
    og = (o.reshape(B, L, DIM) * gate).reshape(R, DIM)

    if _K2 is None:
        _K2 = _build_k2()
    Wo32 = np.asarray(Wo, np.float32)
    in_maps2 = []
    for c in range(NCORES):
        cols = slice(CPC * c, CPC * (c + 1))
        ogT = np.ascontiguousarray(og[:, cols].T).astype(bf16)     # (128, R)
        woT = np.ascontiguousarray(Wo32[:, cols].T).astype(bf16)   # (128, DIM)
        in_maps2.append({"ogT": ogT, "woT": woT})
    res2 = _run(_K2, in_maps2)

    out = np.zeros((R, DIM), np.float32)
    for c in range(NCORES):
        out += np.asarray(res2[c]["par"], np.float32)
    return out.reshape(B, L, DIM)



# revision 2
# speedup vs baseline: 4.6736x; 4.6736x over previous
"""Fused single-NEFF kernel for nn_Atlas_154618823086.

One SPMD program, batch-parallel: core c processes batch c (4 cores).
Everything on device: projections, causal conv+silu, l2norm, the 256-step
fast-weight chunk recurrence (all-f32), layernorm, gating, output matmul.

Layout conventions:
  chan fold: c = g*128 + p  ->  tensors [128, 8, X]
  head of chan c: h = c // 64
  block-diag D-col: 4h + D   (64 cols)
  flat W_out: rows 4h+D (64), cols chan (1024)
"""
import numpy as np
import ml_dtypes
from contextlib import ExitStack

import concourse.tile as tile
import concourse.bass as bass
from concourse import bacc, mybir

f32 = mybir.dt.float32
b16 = mybir.dt.bfloat16
AF = mybir.ActivationFunctionType
ALU = mybir.AluOpType
AX = mybir.AxisListType
bf16 = ml_dtypes.bfloat16

DIM = 1024
H = 16
HD = 64
DI = 4
CH = 16          # chunk length
BASE_LR = 1e-3


def build_fused(L=4096):
    NT = L // 128        # token tiles
    NCK = L // CH        # chunks
    NS = L // 512        # 512-token slices

    nc = bacc.Bacc()
    # ---- inputs ----
    hsT_d = nc.dram_tensor("hsT", [128, 8, L], b16, kind="ExternalInput")
    w4T_d = nc.dram_tensor("w4T", [128, 8, 4096], b16, kind="ExternalInput")
    wlrT_d = nc.dram_tensor("wlrT", [128, 8, 32], b16, kind="ExternalInput")
    w2_d = nc.dram_tensor("w2", [128, 16, 1024], b16, kind="ExternalInput")
    convw_d = nc.dram_tensor("convw", [128, 8, 12], f32, kind="ExternalInput")
    wbdin0_d = nc.dram_tensor("wbdin0", [128, 8, 64], f32, kind="ExternalInput")
    woutT0_d = nc.dram_tensor("woutT0", [128, 8, 64], f32, kind="ExternalInput")
    wout0_d = nc.dram_tensor("wout0", [64, 1024], f32, kind="ExternalInput")
    maskT_d = nc.dram_tensor("maskT", [128, 8, 64], f32, kind="ExternalInput")
    maskF_d = nc.dram_tensor("maskF", [64, 1024], f32, kind="ExternalInput")
    mask16_d = nc.dram_tensor("mask16", [16, 256], f32, kind="ExternalInput")
    e4_d = nc.dram_tensor("e4", [16, 64], f32, kind="ExternalInput")
    e64_d = nc.dram_tensor("e64", [16, 1024], f32, kind="ExternalInput")
    ones2_d = nc.dram_tensor("ones2", [128, 2], f32, kind="ExternalInput")
    onesbT_d = nc.dram_tensor("onesbT", [128, 8, 16], f32, kind="ExternalInput")
    eye_d = nc.dram_tensor("eye", [128, 128], f32, kind="ExternalInput")
    sel_d = nc.dram_tensor("sel", [64, 256], f32, kind="ExternalInput")
    # ---- output ----
    out_d = nc.dram_tensor("out", [L, 1024], b16, kind="ExternalOutput")
    # ---- DRAM scratch ----
    stq_d = nc.dram_tensor("stq", [128, 8, L], f32)   # chan-major pre-norm
    stk_d = nc.dram_tensor("stk", [128, 8, L], f32)
    stv_d = nc.dram_tensor("stv", [128, 8, L], f32)   # final vT (no norm)
    stg_d = nc.dram_tensor("stg", [128, 8, L], f32)   # gate chan-major f32
    qT_d = nc.dram_tensor("qTn", [128, 8, L], f32)    # normalized chan-major
    kT_d = nc.dram_tensor("kTn", [128, 8, L], f32)
    kf_d = nc.dram_tensor("kf", [L, 1024], f32)       # tok-major
    vf_d = nc.dram_tensor("vf", [L, 1024], f32)
    gatef_d = nc.dram_tensor("gatef", [L, 1024], f32)
    gateT_d = nc.dram_tensor("gateT", [128, 8, L], b16)
    obuf_d = nc.dram_tensor("obuf", [L, 1024], f32)
    lrT_d = nc.dram_tensor("lrT", [32, L], f32)
    sqq_d = nc.dram_tensor("sqq", [128, 8, L], f32)
    sqk_d = nc.dram_tensor("sqk", [128, 8, L], f32)

    with tile.TileContext(nc) as tc, ExitStack() as ctx:
        constp = ctx.enter_context(tc.tile_pool(name="const", bufs=1))
        eye = constp.tile([128, 128], f32)
        nc.sync.dma_start(eye[:], eye_d[:])
        eyeb = constp.tile([128, 128], b16)
        nc.vector.tensor_copy(eyeb[:], eye[:])
        e4 = constp.tile([16, 64], f32)
        nc.sync.dma_start(e4[:], e4_d[:])
        e64 = constp.tile([16, 1024], f32)
        nc.sync.dma_start(e64[:], e64_d[:])
        ones2 = constp.tile([128, 2], f32)
        nc.sync.dma_start(ones2[:], ones2_d[:])
        cw0 = constp.tile([128, 8, 12], f32)
        nc.sync.dma_start(cw0[:], convw_d[:])
        c_lr = constp.tile([128, 1], f32)
        nc.vector.memset(c_lr[:], BASE_LR)
        c_eps = constp.tile([128, 1], f32)
        nc.vector.memset(c_eps[:], 1e-5)

        # ================= P1 + P2a: projections, conv, silu =================
        with tc.tile_pool(name="hsp", bufs=1) as hsp, \
             tc.tile_pool(name="p2w", bufs=2) as p2w, \
             tc.tile_pool(name="p2x", bufs=2) as p2x, \
             tc.tile_pool(name="p2y", bufs=1) as p2y, \
             tc.tile_pool(name="p2o", bufs=2) as p2o, \
             tc.tile_pool(name="p2ps", bufs=4, space="PSUM") as p2ps:
            hsT = hsp.tile([128, 8, L], b16)
            nc.sync.dma_start(hsT[:], hsT_d[:])
            wlr = p2w.tile([128, 8, 32], b16, tag="wlr")
            nc.sync.dma_start(wlr[:], wlrT_d[:])
            # lr projections: lrT (32, L) f32, softplus(x + BASE_LR) -> DRAM
            for s in range(NS):
                ps = p2ps.tile([32, 512], f32, tag="lr")
                for kg in range(8):
                    nc.tensor.matmul(ps[:], wlr[:, kg, :],
                                     hsT[:, kg, s * 512:(s + 1) * 512],
                                     start=(kg == 0), stop=(kg == 7))
                # softplus(x + BASE_LR) = ln(1 + exp(x + BASE_LR))
                lre = p2o.tile([32, 512], f32, tag="lre")
                nc.scalar.activation(lre[:], ps[:], AF.Exp, bias=c_lr[0:32, :])
                lrs = p2o.tile([32, 512], f32, tag="lrs")
                nc.scalar.activation(lrs[:], lre[:], AF.Ln, bias=1.0)
                nc.sync.dma_start(lrT_d[:, s * 512:(s + 1) * 512], lrs[:])
            # q/k/v/gate col-tiles
            for ct in range(32):
                j, g = ct // 8, ct % 8
                w4 = p2w.tile([128, 8, 128], b16, tag="w4")
                nc.sync.dma_start(w4[:], w4T_d[:, :, ct * 128:(ct + 1) * 128])
                x = p2x.tile([128, L], f32, tag="x")
                for s in range(NS):
                    ps = p2ps.tile([128, 512], f32, tag="mm")
                    for kg in range(8):
                        nc.tensor.matmul(ps[:], w4[:, kg, :],
                                         hsT[:, kg, s * 512:(s + 1) * 512],
                                         start=(kg == 0), stop=(kg == 7))
                    nc.vector.tensor_copy(x[:, s * 512:(s + 1) * 512], ps[:])
                if j < 3:
                    acc = p2y.tile([128, L], f32, tag="acc")
                    nc.vector.tensor_scalar_mul(acc[:], x[:],
                                                cw0[:, g, 4 * j + 3:4 * j + 4])
                    for sh in (1, 2, 3):
                        nc.vector.scalar_tensor_tensor(
                            acc[:, sh:L], x[:, 0:L - sh],
                            cw0[:, g, 4 * j + (3 - sh):4 * j + (4 - sh)],
                            acc[:, sh:L], op0=ALU.mult, op1=ALU.add)
                    sg = p2y.tile([128, L], f32, tag="sg")
                    nc.scalar.activation(sg[:], acc[:], AF.Sigmoid)
                    nc.vector.tensor_mul(acc[:], acc[:], sg[:])
                    st = (stq_d, stk_d, stv_d)[j]
                    nc.sync.dma_start(st[:, g, :], acc[:])
                    if j < 2:
                        sq = p2y.tile([128, L], f32, tag="sq")
                        nc.vector.tensor_mul(sq[:], acc[:], acc[:])
                        nc.sync.dma_start((sqq_d, sqk_d)[j][:, g, :], sq[:])
                else:
                    nc.sync.dma_start(stg_d[:, g, :], x[:])
                    gb = p2o.tile([128, L], b16, tag="gb")
                    nc.vector.tensor_copy(gb[:], x[:])
                    nc.sync.dma_start(gateT_d[:, g, :], gb[:])

        # ================= P2n: l2 norms (rno = 1/||.||) ====================
        # n2[h, tok] = sum_c onesbT[c, h] * sq[c, tok] via chained matmuls
        rnos = {}
        with tc.tile_pool(name="nrm", bufs=3) as nrm, \
             tc.tile_pool(name="rnop", bufs=1) as rnop, \
             tc.tile_pool(name="nps", bufs=4, space="PSUM") as nps:
            onesbT = nrm.tile([128, 8, 16], f32, tag="onesbT")
            nc.sync.dma_start(onesbT[:], onesbT_d[:])
            for name, sq_d in (("q", sqq_d), ("k", sqk_d)):
                rno = rnop.tile([16, L], f32, tag=f"rno_{name}")
                rnos[name] = rno
                for s in range(NS):
                    sqs = nrm.tile([128, 8, 512], f32, tag="sqs")
                    nc.sync.dma_start(sqs[:], sq_d[:, :, s * 512:(s + 1) * 512])
                    ps = nps.tile([16, 512], f32, tag="n2")
                    for g in range(8):
                        nc.tensor.matmul(ps[:], onesbT[:, g, :], sqs[:, g, :],
                                         start=(g == 0), stop=(g == 7))
                    nrm_t = nrm.tile([16, 512], f32, tag="nrm_t")
                    nc.scalar.activation(nrm_t[:], ps[:], AF.Sqrt)
                    nc.vector.reciprocal(rno[:, s * 512:(s + 1) * 512], nrm_t[:])

        # ================= P2c: normalize q,k chan-major ====================
        with tc.tile_pool(name="c2", bufs=2) as c2p, \
             tc.tile_pool(name="c2ps", bufs=4, space="PSUM") as c2ps:
            for name, st, dst in (("q", stq_d, qT_d), ("k", stk_d, kT_d)):
                rno = rnos[name]
                for g in range(8):
                    xin = c2p.tile([128, L], f32, tag="xin")
                    nc.sync.dma_start(xin[:], st[:, g, :])
                    xo = c2p.tile([128, L], f32, tag="xo")
                    for s in range(NS):
                        ps = c2ps.tile([128, 512], f32, tag="bc")
                        nc.tensor.matmul(ps[:], e64[:, g * 128:(g + 1) * 128],
                                         rno[:, s * 512:(s + 1) * 512],
                                         start=True, stop=True)
                        nc.vector.tensor_mul(xo[:, s * 512:(s + 1) * 512],
                                             xin[:, s * 512:(s + 1) * 512], ps[:])
                    nc.sync.dma_start(dst[:, g, :], xo[:])

        # ================= P2b: transposes to tok-major =====================
        with tc.tile_pool(name="tb", bufs=3) as tbp, \
             tc.tile_pool(name="tbps", bufs=4, space="PSUM") as tbps:
            for src, dst, dt_ in ((kT_d, kf_d, f32), (stv_d, vf_d, f32),
                                  (stg_d, gatef_d, f32)):
                for t in range(NT):
                    xin = tbp.tile([128, 8, 128], f32, tag="xin")
                    nc.sync.dma_start(xin[:], src[:, :, t * 128:(t + 1) * 128])
                    xo = tbp.tile([128, 1024], dt_, tag="xo")
                    for g in range(8):
                        ps = tbps.tile([128, 128], f32, tag="tp")
                        nc.tensor.transpose(ps[:], xin[:, g, :], eye[:])
                        nc.vector.tensor_copy(xo[:, g * 128:(g + 1) * 128], ps[:])
                    nc.sync.dma_start(dst[t * 128:(t + 1) * 128, :], xo[:])

        # ================= P2R: fast-weight recurrence ======================
        win = constp.tile([128, 8, 64], f32)      # block-diag W_in  (chan, 4h+D)
        nc.sync.dma_start(win[:], wbdin0_d[:])
        woutT = constp.tile([128, 8, 64], f32)    # W_out^T block-diag
        nc.sync.dma_start(woutT[:], woutT0_d[:])
        wout = constp.tile([64, 1024], f32)       # W_out flat (4h+D, chan)
        nc.sync.dma_start(wout[:], wout0_d[:])
        maskT = constp.tile([128, 8, 64], f32)
        nc.sync.dma_start(maskT[:], maskT_d[:])
        maskF = constp.tile([64, 1024], f32)
        nc.sync.dma_start(maskF[:], maskF_d[:])
        mask16T = constp.tile([16, 256], f32)
        nc.sync.dma_start(mask16T[:], mask16_d[:])
        sel = constp.tile([64, 256], f32)
        nc.sync.dma_start(sel[:], sel_d[:])

        with tc.tile_pool(name="rin", bufs=3) as rin, \
             tc.tile_pool(name="rw", bufs=2) as rw, \
             tc.tile_pool(name="rps", bufs=2, space="PSUM") as rps, \
             tc.tile_pool(name="rpo", bufs=2, space="PSUM") as rpo, \
             tc.tile_pool(name="rpu", bufs=1, space="PSUM") as rpu:

            def softmax4(s_ps, tag):
                # s_ps (16, 64) = scores grouped [h, D]; softmax over D (4)
                nmax = rw.tile([16, 16], f32, tag=f"nm_{tag}")
                nc.vector.tensor_reduce(
                    nmax[:], s_ps[:].rearrange("p (g x) -> p g x", x=4),
                    axis=AX.X, op=ALU.max, negate=True)
                e = rw.tile([16, 64], f32, tag=f"e_{tag}")
                nc.vector.tensor_tensor(
                    e[:].rearrange("p (g x) -> p g x", x=4),
                    s_ps[:].rearrange("p (g x) -> p g x", x=4),
                    nmax[:, :, None].broadcast_to([16, 16, 4]), op=ALU.add)
                nc.scalar.activation(e[:], e[:], AF.Exp)
                gs = rw.tile([16, 16], f32, tag=f"gs_{tag}")
                nc.vector.tensor_reduce(
                    gs[:], e[:].rearrange("p (g x) -> p g x", x=4),
                    axis=AX.X, op=ALU.add)
                gr = rw.tile([16, 16], f32, tag=f"gr_{tag}")
                nc.vector.reciprocal(gr[:], gs[:])
                p = rw.tile([16, 64], f32, tag=f"p_{tag}")
                nc.vector.tensor_tensor(
                    p[:].rearrange("p (g x) -> p g x", x=4),
                    e[:].rearrange("p (g x) -> p g x", x=4),
                    gr[:, :, None].broadcast_to([16, 16, 4]), op=ALU.mult)
                return p

            def softmax16(s_ps, tag):
                # s_ps (64, 16): softmax over free dim of s/8
                nmax = rw.tile([64, 1], f32, tag=f"nm16_{tag}")
                nc.vector.tensor_reduce(nmax[:], s_ps[:], axis=AX.X,
                                        op=ALU.max, negate=True)
                nm8 = rw.tile([64, 1], f32, tag=f"nm8_{tag}")
                nc.vector.tensor_scalar_mul(nm8[:], nmax[:], 0.125)
                e = rw.tile([64, 16], f32, tag=f"e16_{tag}")
                nc.scalar.activation(e[:], s_ps[:], AF.Exp,
                                     bias=nm8[:], scale=0.125)
                rs = rw.tile([64, 1], f32, tag=f"rs_{tag}")
                nc.vector.tensor_reduce(rs[:], e[:], axis=AX.X, op=ALU.add)
                rr = rw.tile([64, 1], f32, tag=f"rr_{tag}")
                nc.vector.reciprocal(rr[:], rs[:])
                p = rw.tile([64, 16], f32, tag=f"p16_{tag}")
                nc.vector.tensor_scalar_mul(p[:], e[:], rr[:])
                return p

            def transpose_to(p_sb, P, Fr, tag):
                # (P, Fr) -> (Fr, P), Fr <= 128
                ps = rps.tile([Fr, P], f32, tag="tp")
                nc.tensor.transpose(ps[:], p_sb[:], eye[:P, :P])
                sb = rw.tile([Fr, P], f32, tag=f"tps_{tag}")
                nc.vector.tensor_copy(sb[:], ps[:])
                return sb

            with tc.For_i(0, NCK, 1) as i:
                t0 = i * CH
                KT = rin.tile([128, 8, CH], f32, tag="KT")
                nc.sync.dma_start(KT[:], kT_d[:, :, bass.ds(t0, CH)])
                QT = rin.tile([128, 8, CH], f32, tag="QT")
                nc.sync.dma_start(QT[:], qT_d[:, :, bass.ds(t0, CH)])
                VT = rin.tile([128, 8, CH], f32, tag="VT")
                nc.sync.dma_start(VT[:], stv_d[:, :, bass.ds(t0, CH)])
                Kf = rin.tile([CH, 1024], f32, tag="Kf")
                nc.sync.dma_start(Kf[:], kf_d[bass.ds(t0, CH), :])
                Vf = rin.tile([CH, 1024], f32, tag="Vf")
                nc.sync.dma_start(Vf[:], vf_d[bass.ds(t0, CH), :])
                lrc1 = rin.tile([16, CH], f32, tag="lrc1")
                nc.sync.dma_start(lrc1[:], lrT_d[0:16, bass.ds(t0, CH)])
                lrc0 = rin.tile([16, CH], f32, tag="lrc0")
                nc.sync.dma_start(lrc0[:], lrT_d[16:32, bass.ds(t0, CH)])

                # --- scores vs W_in, chunk-local attention ---
                sk_ps = rps.tile([16, 64], f32, tag="s")
                for g in range(8):
                    nc.tensor.matmul(sk_ps[:], KT[:, g, :], win[:, g, :],
                                     start=(g == 0), stop=(g == 7))
                p_k = softmax4(sk_ps, "k")
                # lr1 broadcast (16, 64): lhsT = lrT[0:16, chunk] (16,16)
                lr1_ps = rps.tile([16, 64], f32, tag="s")
                nc.tensor.matmul(lr1_ps[:], lrc1[:], e4[:],
                                 start=True, stop=True)
                k_h = rw.tile([16, 64], f32, tag="k_h")
                nc.vector.tensor_mul(k_h[:], p_k[:], lr1_ps[:])

                sq_ps = rps.tile([16, 64], f32, tag="s")
                for g in range(8):
                    nc.tensor.matmul(sq_ps[:], QT[:, g, :], win[:, g, :],
                                     start=(g == 0), stop=(g == 7))
                q_h = softmax4(sq_ps, "q")

                q_hT = transpose_to(q_h, 16, 64, "qh")
                k_hT = transpose_to(k_h, 16, 64, "kh")

                # block-diagonal expansion: q_hX = SEL * tile16(q_hT)
                q_hX = rw.tile([64, 256], f32, tag="q_hX")
                nc.vector.tensor_tensor(
                    q_hX[:].rearrange("p (h q) -> p h q", q=16),
                    sel[:].rearrange("p (h q) -> p h q", q=16),
                    q_hT[:, None, :].broadcast_to([64, 16, 16]),
                    op=ALU.mult)
                # S_catT[k', 16h+q] = sum_D k_h[k', 4h+D] q_h[q, 4h+D]
                ST_ps = rps.tile([16, 256], f32, tag="s")
                nc.tensor.matmul(ST_ps[:], k_hT[:], q_hX[:],
                                 start=True, stop=True)
                S_mT = rw.tile([16, 256], f32, tag="S_mT")
                nc.vector.tensor_mul(S_mT[:], ST_ps[:], mask16T[:])

                # o = q_h @ W_out + S_mT-applied V  (two 512-col halves)
                o_sb = rw.tile([16, 1024], f32, tag="o_sb")
                for half in range(2):
                    o_ps = rpo.tile([16, 512], f32, tag="o")
                    nc.tensor.matmul(o_ps[:], q_hT[:],
                                     wout[:, half * 512:(half + 1) * 512],
                                     start=True, stop=False)
                    for hh in range(8):
                        h = half * 8 + hh
                        nc.tensor.matmul(
                            o_ps[:, hh * 64:(hh + 1) * 64],
                            S_mT[:, 16 * h:16 * (h + 1)],
                            Vf[:, h * 64:(h + 1) * 64],
                            start=False, stop=(hh == 7))
                    nc.vector.tensor_copy(o_sb[:, half * 512:(half + 1) * 512],
                                          o_ps[:])
                nc.sync.dma_start(obuf_d[bass.ds(t0, CH), :], o_sb[:])

                # --- W_out += k_h^T @ V (flat + transposed) ---
                for half in range(2):
                    u_ps = rpu.tile([64, 512], f32, tag="u")
                    nc.tensor.matmul(u_ps[:], k_h[:],
                                     Vf[:, half * 512:(half + 1) * 512],
                                     start=True, stop=True)
                    tmp = rw.tile([64, 512], f32, tag="uf")
                    nc.vector.tensor_mul(tmp[:], u_ps[:],
                                         maskF[:, half * 512:(half + 1) * 512])
                    nc.vector.tensor_add(wout[:, half * 512:(half + 1) * 512],
                                         wout[:, half * 512:(half + 1) * 512],
                                         tmp[:])
                uT_ps = rpu.tile([128, 8, 64], f32, tag="uT")
                for g in range(8):
                    nc.tensor.matmul(uT_ps[:, g, :],
                                     Vf[:, g * 128:(g + 1) * 128], k_h[:],
                                     start=True, stop=True)
                tmpT = rw.tile([128, 8, 64], f32, tag="uTf")
                nc.vector.tensor_mul(tmpT[:], uT_ps[:], maskT[:])
                nc.vector.tensor_add(woutT[:], woutT[:], tmpT[:])

                # lr columns for this chunk
                lrin_ps = rps.tile([128, 8], f32, tag="s")
                lrout_ps = rps.tile([128, 8], f32, tag="tp")
                for g in range(8):
                    nc.tensor.matmul(lrin_ps[:, g:g + 1],
                                     e64[:, g * 128:(g + 1) * 128],
                                     lrc0[:, 0:1], start=True, stop=True)
                    nc.tensor.matmul(lrout_ps[:, g:g + 1],
                                     e64[:, g * 128:(g + 1) * 128],
                                     lrc1[:, 0:1], start=True, stop=True)
                lrin_b = rw.tile([128, 8], f32, tag="lrin_b")
                nc.vector.tensor_copy(lrin_b[:], lrin_ps[:])
                lrout_b = rw.tile([128, 8], f32, tag="lrout_b")
                nc.vector.tensor_copy(lrout_b[:], lrout_ps[:])
                lroutD_ps = rps.tile([64, 1], f32, tag="s")
                nc.tensor.matmul(lroutD_ps[:], e4[:], lrc1[:, 0:1],
                                 start=True, stop=True)
                lroutD = rw.tile([64, 1], f32, tag="lroutD")
                nc.vector.tensor_copy(lroutD[:], lroutD_ps[:])

                # --- two test-time gradient steps ---
                for it in range(2):
                    # g_out = attn(W_in, K, V): S1 = Wbd_in^T @ KT
                    S1_ps = rps.tile([64, 16], f32, tag="s")
                    for g in range(8):
                        nc.tensor.matmul(S1_ps[:], win[:, g, :], KT[:, g, :],
                                         start=(g == 0), stop=(g == 7))
                    p1 = softmax16(S1_ps, "p1")
                    p1T = transpose_to(p1, 64, 16, "p1")
                    # W_out += lroutD * maskF * (p1 @ Vf) ; halves
                    for half in range(2):
                        g1_ps = rpu.tile([64, 512], f32, tag="u")
                        nc.tensor.matmul(g1_ps[:], p1T[:],
                                         Vf[:, half * 512:(half + 1) * 512],
                                         start=True, stop=True)
                        tmp = rw.tile([64, 512], f32, tag="uf")
                        nc.vector.tensor_mul(tmp[:], g1_ps[:],
                                             maskF[:, half * 512:(half + 1) * 512])
                        nc.vector.scalar_tensor_tensor(
                            wout[:, half * 512:(half + 1) * 512], tmp[:],
                            lroutD[:],
                            wout[:, half * 512:(half + 1) * 512],
                            op0=ALU.mult, op1=ALU.add)
                    # transposed copy
                    g1T_ps = rpu.tile([128, 8, 64], f32, tag="uT")
                    for g in range(8):
                        nc.tensor.matmul(g1T_ps[:, g, :],
                                         Vf[:, g * 128:(g + 1) * 128], p1T[:],
                                         start=True, stop=True)
                    g1T = rw.tile([128, 8, 64], f32, tag="uTf")
                    nc.vector.tensor_mul(g1T[:], g1T_ps[:], maskT[:])
                    for g in range(8):
                        nc.vector.scalar_tensor_tensor(
                            woutT[:, g, :], g1T[:, g, :], lrout_b[:, g:g + 1],
                            woutT[:, g, :], op0=ALU.mult, op1=ALU.add)

                    # g_in = attn(W_out, V, K): S2 = W_outT^T @ VT
                    S2_ps = rps.tile([64, 16], f32, tag="s")
                    for g in range(8):
                        nc.tensor.matmul(S2_ps[:], woutT[:, g, :], VT[:, g, :],
                                         start=(g == 0), stop=(g == 7))
                    p2 = softmax16(S2_ps, "p2")
                    p2T = transpose_to(p2, 64, 16, "p2")
                    # W_in += lrin_b * maskT * (Kf^T-tiled @ p2T)
                    g2_ps = rpu.tile([128, 8, 64], f32, tag="uT")
                    for g in range(8):
                        nc.tensor.matmul(g2_ps[:, g, :],
                                         Kf[:, g * 128:(g + 1) * 128], p2T[:],
                                         start=True, stop=True)
                    g2 = rw.tile([128, 8, 64], f32, tag="uTf")
                    nc.vector.tensor_mul(g2[:], g2_ps[:], maskT[:])
                    for g in range(8):
                        nc.vector.scalar_tensor_tensor(
                            win[:, g, :], g2[:, g, :], lrin_b[:, g:g + 1],
                            win[:, g, :], op0=ALU.mult, op1=ALU.add)

        # ================= P3: layernorm, gate, out matmul ==================
        with tc.tile_pool(name="f3", bufs=2) as f3p, \
             tc.tile_pool(name="f3w", bufs=1) as f3w, \
             tc.tile_pool(name="f3ps", bufs=4, space="PSUM") as f3ps, \
             tc.tile_pool(name="f3po", bufs=2, space="PSUM") as f3po:
            w2 = f3w.tile([128, 16, 1024], b16)
            nc.sync.dma_start(w2[:], w2_d[:])
            for t in range(NT):
                o = f3p.tile([128, 1024], f32, tag="o")
                nc.sync.dma_start(o[:], obuf_d[t * 128:(t + 1) * 128, :])
                gf = f3p.tile([128, 1024], f32, tag="gf")
                nc.sync.dma_start(gf[:], gatef_d[t * 128:(t + 1) * 128, :])
                gT = f3p.tile([128, 8, 128], b16, tag="gT")
                nc.sync.dma_start(gT[:], gateT_d[:, :, t * 128:(t + 1) * 128])
                # LN over 64-groups
                ssum = f3p.tile([128, 16], f32, tag="ssum")
                nc.vector.tensor_reduce(
                    ssum[:], o[:].rearrange("p (g x) -> p g x", x=64),
                    axis=AX.X, op=ALU.add)
                mu = f3p.tile([128, 16], f32, tag="mu")
                nc.vector.tensor_scalar_mul(mu[:], ssum[:], 1.0 / 64)
                xm = f3p.tile([128, 1024], f32, tag="xm")
                nc.vector.tensor_tensor(
                    xm[:].rearrange("p (g x) -> p g x", x=64),
                    o[:].rearrange("p (g x) -> p g x", x=64),
                    mu[:, :, None].broadcast_to([128, 16, 64]), op=ALU.subtract)
                sq2 = f3p.tile([128, 1024], f32, tag="sq2")
                nc.vector.tensor_mul(sq2[:], xm[:], xm[:])
                var = f3p.tile([128, 16], f32, tag="var")
                nc.vector.tensor_reduce(
                    var[:], sq2[:].rearrange("p (g x) -> p g x", x=64),
                    axis=AX.X, op=ALU.add)
                sd = f3p.tile([128, 16], f32, tag="sd")
                nc.scalar.activation(sd[:], var[:], AF.Sqrt,
                                     bias=c_eps[:], scale=1.0 / 64)
                rsd = f3p.tile([128, 16], f32, tag="rsd")
                nc.vector.reciprocal(rsd[:], sd[:])
                xn = f3p.tile([128, 1024], f32, tag="xn")
                nc.vector.tensor_tensor(
                    xn[:].rearrange("p (g x) -> p g x", x=64),
                    xm[:].rearrange("p (g x) -> p g x", x=64),
                    rsd[:, :, None].broadcast_to([128, 16, 64]), op=ALU.mult)
                xg = f3p.tile([128, 1024], b16, tag="xg")
                nc.vector.tensor_mul(xg[:], xn[:], gf[:])
                # transpose xg to (chan, tok) tiles
                xgT = f3p.tile([128, 8, 128], b16, tag="xgT")
                for g in range(8):
                    ps = f3ps.tile([128, 128], b16, tag="tp")
                    nc.tensor.transpose(ps[:], xg[:, g * 128:(g + 1) * 128],
                                        eyeb[:])
                    nc.vector.tensor_copy(xgT[:, g, :], ps[:])
                oo = f3p.tile([128, 1024], b16, tag="oo")
                for half in range(2):
                    ps = f3po.tile([128, 512], f32, tag="out")
                    for kg in range(16):
                        lhsT = xgT[:, kg, :] if kg < 8 else gT[:, kg - 8, :]
                        nc.tensor.matmul(ps[:], lhsT,
                                         w2[:, kg, half * 512:(half + 1) * 512],
                                         start=(kg == 0), stop=(kg == 15))
                    nc.vector.tensor_copy(oo[:, half * 512:(half + 1) * 512],
                                          ps[:])
                nc.sync.dma_start(out_d[t * 128:(t + 1) * 128, :], oo[:])

    nc.compile()
    return nc


# ======================= host-side preparation =============================

def fold(M):
    """(1024, X) -> (128, 8, X) with chan = g*128 + p."""
    return np.ascontiguousarray(
        np.asarray(M).reshape(8, 128, -1).transpose(1, 0, 2))


def prep_weights(Wq, Wk, Wv, Wlr, Wg, Wo, cq, ck, cv, W_in_init, W_out_init,
                 ln_g, ln_b):
    """Build the shared (batch-independent) input tensors."""
    W = {}
    w4 = np.concatenate([np.asarray(x, np.float32).T
                         for x in (Wq, Wk, Wv, Wg)], axis=1)   # (1024, 4096)
    W["w4T"] = fold(w4).astype(bf16)
    perm = [2 * h + 1 for h in range(16)] + [2 * h for h in range(16)]
    W["wlrT"] = fold(np.asarray(Wlr, np.float32)[perm].T).astype(bf16)
    lng = np.tile(np.asarray(ln_g, np.float32), 16)
    lnb = np.tile(np.asarray(ln_b, np.float32), 16)
    WoT = np.asarray(Wo, np.float32).T                          # (chan, out)
    W2 = np.concatenate([lng[:, None] * WoT, lnb[:, None] * WoT], axis=0)
    W["w2"] = np.ascontiguousarray(
        W2.reshape(16, 128, 1024).transpose(1, 0, 2)).astype(bf16)
    convw = np.zeros((1024, 12), np.float32)
    for j, cw in enumerate((cq, ck, cv)):
        convw[:, 4 * j:4 * j + 4] = np.asarray(cw, np.float32)
        convw[:, 4 * j + 3] += 1.0
    W["convw"] = fold(convw)
    Win0 = np.asarray(W_in_init, np.float32)[0]    # (4, 16, 64)
    Wout0 = np.asarray(W_out_init, np.float32)[0]
    wbdin0 = np.zeros((1024, 64), np.float32)
    woutT0 = np.zeros((1024, 64), np.float32)
    maskT = np.zeros((1024, 64), np.float32)
    for h in range(16):
        for d in range(64):
            c = 64 * h + d
            wbdin0[c, 4 * h:4 * h + 4] = Win0[:, h, d]
            woutT0[c, 4 * h:4 * h + 4] = Wout0[:, h, d]
            maskT[c, 4 * h:4 * h + 4] = 1.0
    W["wbdin0"] = fold(wbdin0)
    W["woutT0"] = fold(woutT0)
    W["maskT"] = fold(maskT)
    wout0 = np.zeros((64, 1024), np.float32)
    maskF = np.zeros((64, 1024), np.float32)
    for h in range(16):
        for D in range(4):
            wout0[4 * h + D, 64 * h:64 * h + 64] = Wout0[D, h]
            maskF[4 * h + D, 64 * h:64 * h + 64] = 1.0
    W["wout0"] = wout0
    W["maskF"] = maskF
    mask16 = np.zeros((16, 256), np.float32)
    trilT = np.tril(np.ones((16, 16), np.float32)).T   # [k', q] = 1 if k' <= q
    for h in range(16):
        mask16[:, 16 * h:16 * h + 16] = trilT
    W["mask16"] = mask16
    e4 = np.zeros((16, 64), np.float32)
    e64 = np.zeros((16, 1024), np.float32)
    for h in range(16):
        e4[h, 4 * h:4 * h + 4] = 1.0
        e64[h, 64 * h:64 * h + 64] = 1.0
    W["e4"] = e4
    W["e64"] = e64
    ones2 = np.zeros((128, 2), np.float32)
    ones2[:64, 0] = 1.0
    ones2[64:, 1] = 1.0
    W["ones2"] = ones2
    onesbT = np.zeros((1024, 16), np.float32)
    for c in range(1024):
        onesbT[c, c // 64] = 1.0
    W["onesbT"] = fold(onesbT)
    sel = np.zeros((64, 16, 16), np.float32)
    for h in range(16):
        sel[4 * h:4 * h + 4, h, :] = 1.0
    W["sel"] = sel.reshape(64, 256)
    W["eye"] = np.eye(128, dtype=np.float32)
    return W


def make_in_map(hs_b, W):
    """Per-core inputs: hs_b (L, 1024) f32 for this core's batch."""
    m = dict(W)
    m["hsT"] = fold(np.asarray(hs_b, np.float32).T).astype(bf16)
    return m


# ======================= kernel entry point ================================

_NC = None
LAST_EXEC_NS = []
NCORES = 4
B = 4
L = 4096


def _run(nc, in_maps):
    import time
    from concourse.bass_utils import run_bass_kernel_spmd
    t0 = time.perf_counter()
    res = run_bass_kernel_spmd(nc, in_maps, core_ids=list(range(len(in_maps))))
    dt = time.perf_counter() - t0
    if res.exec_time_ns is not None:
        LAST_EXEC_NS.append(res.exec_time_ns)
    else:
        LAST_EXEC_NS.append(int(dt * 1e9))
    return res.results


def kernel(hidden_states, Wq, Wk, Wv, Wlr, Wg, Wo, cq, ck, cv,
           W_in_init, W_out_init, ln_g, ln_b):
    global _NC
    if _NC is None:
        _NC = build_fused(L=L)
    W = prep_weights(Wq, Wk, Wv, Wlr, Wg, Wo, cq, ck, cv,
                     W_in_init, W_out_init, ln_g, ln_b)
    hs = np.asarray(hidden_states, np.float32)
    in_maps = [make_in_map(hs[b], W) for b in range(B)]
    results = _run(_NC, in_maps)
    out = np.stack([np.asarray(results[b]["out"], np.float32)
                    for b in range(B)])
    return out


# revision 3
# speedup vs baseline: 7.2735x; 1.5563x over previous
"""Fused single-NEFF kernel for nn_Atlas_154618823086.

One SPMD program, batch-parallel: core c processes batch c (4 cores).
Everything on device: projections, causal conv+silu, l2norm, the 256-step
fast-weight chunk recurrence (all-f32), layernorm, gating, output matmul.

Layout conventions:
  chan fold: c = g*128 + p  ->  tensors [128, 8, X]
  head of chan c: h = c // 64
  block-diag D-col: 4h + D   (64 cols)
  flat W_out: rows 4h+D (64), cols chan (1024)
"""
import numpy as np
import ml_dtypes
from contextlib import ExitStack

import concourse.tile as tile
import concourse.bass as bass
from concourse import bacc, mybir

f32 = mybir.dt.float32
b16 = mybir.dt.bfloat16
AF = mybir.ActivationFunctionType
ALU = mybir.AluOpType
AX = mybir.AxisListType
bf16 = ml_dtypes.bfloat16

DIM = 1024
H = 16
HD = 64
DI = 4
CH = 16          # chunk length
BASE_LR = 1e-3


def build_fused(L=4096):
    NT = L // 128        # token tiles
    NCK = L // CH        # chunks
    NS = L // 512        # 512-token slices

    nc = bacc.Bacc()
    # ---- inputs ----
    hsT_d = nc.dram_tensor("hsT", [128, 8, L], b16, kind="ExternalInput")
    w4T_d = nc.dram_tensor("w4T", [128, 8, 4096], b16, kind="ExternalInput")
    wlrT_d = nc.dram_tensor("wlrT", [128, 8, 32], b16, kind="ExternalInput")
    w2_d = nc.dram_tensor("w2", [128, 16, 1024], b16, kind="ExternalInput")
    convw_d = nc.dram_tensor("convw", [128, 8, 12], f32, kind="ExternalInput")
    ain_d = nc.dram_tensor("ain", [128, 8, 4], f32, kind="ExternalInput")
    aoutT_d = nc.dram_tensor("aoutT", [128, 8, 4], f32, kind="ExternalInput")
    bout_d = nc.dram_tensor("bout", [64, 64], f32, kind="ExternalInput")
    maskT_d = nc.dram_tensor("maskT", [128, 8, 64], f32, kind="ExternalInput")
    maskF_d = nc.dram_tensor("maskF", [64, 1024], f32, kind="ExternalInput")
    mask16_d = nc.dram_tensor("mask16", [16, 256], f32, kind="ExternalInput")
    e4_d = nc.dram_tensor("e4", [16, 64], f32, kind="ExternalInput")
    e64_d = nc.dram_tensor("e64", [16, 1024], f32, kind="ExternalInput")
    ones2_d = nc.dram_tensor("ones2", [128, 2], f32, kind="ExternalInput")
    onesbT_d = nc.dram_tensor("onesbT", [128, 8, 16], f32, kind="ExternalInput")
    eye_d = nc.dram_tensor("eye", [128, 128], f32, kind="ExternalInput")
    sel_d = nc.dram_tensor("sel", [64, 256], f32, kind="ExternalInput")
    # ---- output ----
    out_d = nc.dram_tensor("out", [L, 1024], b16, kind="ExternalOutput")
    # ---- DRAM scratch ----
    stq_d = nc.dram_tensor("stq", [128, 8, L], f32)   # chan-major pre-norm
    stk_d = nc.dram_tensor("stk", [128, 8, L], f32)
    stv_d = nc.dram_tensor("stv", [128, 8, L], f32)   # final vT (no norm)
    stg_d = nc.dram_tensor("stg", [128, 8, L], f32)   # gate chan-major f32
    qT_d = nc.dram_tensor("qTn", [128, 8, L], f32)    # normalized chan-major
    kT_d = nc.dram_tensor("kTn", [128, 8, L], f32)
    kf_d = nc.dram_tensor("kf", [L, 1024], f32)       # tok-major
    vf_d = nc.dram_tensor("vf", [L, 1024], f32)
    gatef_d = nc.dram_tensor("gatef", [L, 1024], f32)
    gateT_d = nc.dram_tensor("gateT", [128, 8, L], b16)
    obuf_d = nc.dram_tensor("obuf", [L, 1024], f32)
    lrT_d = nc.dram_tensor("lrT", [32, L], f32)
    sqq_d = nc.dram_tensor("sqq", [128, 8, L], f32)
    sqk_d = nc.dram_tensor("sqk", [128, 8, L], f32)

    with tile.TileContext(nc) as tc, ExitStack() as ctx:
        constp = ctx.enter_context(tc.tile_pool(name="const", bufs=1))
        eye = constp.tile([128, 128], f32)
        nc.sync.dma_start(eye[:], eye_d[:])
        eyeb = constp.tile([128, 128], b16)
        nc.vector.tensor_copy(eyeb[:], eye[:])
        e4 = constp.tile([16, 64], f32)
        nc.sync.dma_start(e4[:], e4_d[:])
        e64 = constp.tile([16, 1024], f32)
        nc.sync.dma_start(e64[:], e64_d[:])
        ones2 = constp.tile([128, 2], f32)
        nc.sync.dma_start(ones2[:], ones2_d[:])
        cw0 = constp.tile([128, 8, 12], f32)
        nc.sync.dma_start(cw0[:], convw_d[:])
        c_lr = constp.tile([128, 1], f32)
        nc.vector.memset(c_lr[:], BASE_LR)
        c_eps = constp.tile([128, 1], f32)
        nc.vector.memset(c_eps[:], 1e-5)

        # ================= P1 + P2a: projections, conv, silu =================
        with tc.tile_pool(name="hsp", bufs=1) as hsp, \
             tc.tile_pool(name="p2w", bufs=2) as p2w, \
             tc.tile_pool(name="p2x", bufs=2) as p2x, \
             tc.tile_pool(name="p2y", bufs=1) as p2y, \
             tc.tile_pool(name="p2o", bufs=2) as p2o, \
             tc.tile_pool(name="p2ps", bufs=4, space="PSUM") as p2ps:
            hsT = hsp.tile([128, 8, L], b16)
            nc.sync.dma_start(hsT[:], hsT_d[:])
            wlr = p2w.tile([128, 8, 32], b16, tag="wlr")
            nc.sync.dma_start(wlr[:], wlrT_d[:])
            # lr projections: lrT (32, L) f32, softplus(x + BASE_LR) -> DRAM
            for s in range(NS):
                ps = p2ps.tile([32, 512], f32, tag="lr")
                for kg in range(8):
                    nc.tensor.matmul(ps[:], wlr[:, kg, :],
                                     hsT[:, kg, s * 512:(s + 1) * 512],
                                     start=(kg == 0), stop=(kg == 7))
                # softplus(x + BASE_LR) = ln(1 + exp(x + BASE_LR))
                lre = p2o.tile([32, 512], f32, tag="lre")
                nc.scalar.activation(lre[:], ps[:], AF.Exp, bias=c_lr[0:32, :])
                lrs = p2o.tile([32, 512], f32, tag="lrs")
                nc.scalar.activation(lrs[:], lre[:], AF.Ln, bias=1.0)
                nc.sync.dma_start(lrT_d[:, s * 512:(s + 1) * 512], lrs[:])
            # q/k/v/gate col-tiles
            for ct in range(32):
                j, g = ct // 8, ct % 8
                w4 = p2w.tile([128, 8, 128], b16, tag="w4")
                nc.sync.dma_start(w4[:], w4T_d[:, :, ct * 128:(ct + 1) * 128])
                x = p2x.tile([128, L], f32, tag="x")
                for s in range(NS):
                    ps = p2ps.tile([128, 512], f32, tag="mm")
                    for kg in range(8):
                        nc.tensor.matmul(ps[:], w4[:, kg, :],
                                         hsT[:, kg, s * 512:(s + 1) * 512],
                                         start=(kg == 0), stop=(kg == 7))
                    nc.vector.tensor_copy(x[:, s * 512:(s + 1) * 512], ps[:])
                if j < 3:
                    acc = p2y.tile([128, L], f32, tag="acc")
                    nc.vector.tensor_scalar_mul(acc[:], x[:],
                                                cw0[:, g, 4 * j + 3:4 * j + 4])
                    for sh in (1, 2, 3):
                        nc.vector.scalar_tensor_tensor(
                            acc[:, sh:L], x[:, 0:L - sh],
                            cw0[:, g, 4 * j + (3 - sh):4 * j + (4 - sh)],
                            acc[:, sh:L], op0=ALU.mult, op1=ALU.add)
                    sg = p2y.tile([128, L], f32, tag="sg")
                    nc.scalar.activation(sg[:], acc[:], AF.Sigmoid)
                    nc.vector.tensor_mul(acc[:], acc[:], sg[:])
                    st = (stq_d, stk_d, stv_d)[j]
                    nc.sync.dma_start(st[:, g, :], acc[:])
                    if j < 2:
                        sq = p2y.tile([128, L], f32, tag="sq")
                        nc.vector.tensor_mul(sq[:], acc[:], acc[:])
                        nc.sync.dma_start((sqq_d, sqk_d)[j][:, g, :], sq[:])
                else:
                    nc.sync.dma_start(stg_d[:, g, :], x[:])
                    gb = p2o.tile([128, L], b16, tag="gb")
                    nc.vector.tensor_copy(gb[:], x[:])
                    nc.sync.dma_start(gateT_d[:, g, :], gb[:])

        # ================= P2n: l2 norms (rno = 1/||.||) ====================
        # n2[h, tok] = sum_c onesbT[c, h] * sq[c, tok] via chained matmuls
        rnos = {}
        with tc.tile_pool(name="nrm", bufs=3) as nrm, \
             tc.tile_pool(name="rnop", bufs=1) as rnop, \
             tc.tile_pool(name="nps", bufs=4, space="PSUM") as nps:
            onesbT = nrm.tile([128, 8, 16], f32, tag="onesbT")
            nc.sync.dma_start(onesbT[:], onesbT_d[:])
            for name, sq_d in (("q", sqq_d), ("k", sqk_d)):
                rno = rnop.tile([16, L], f32, tag=f"rno_{name}")
                rnos[name] = rno
                for s in range(NS):
                    sqs = nrm.tile([128, 8, 512], f32, tag="sqs")
                    nc.sync.dma_start(sqs[:], sq_d[:, :, s * 512:(s + 1) * 512])
                    ps = nps.tile([16, 512], f32, tag="n2")
                    for g in range(8):
                        nc.tensor.matmul(ps[:], onesbT[:, g, :], sqs[:, g, :],
                                         start=(g == 0), stop=(g == 7))
                    nrm_t = nrm.tile([16, 512], f32, tag="nrm_t")
                    nc.scalar.activation(nrm_t[:], ps[:], AF.Sqrt)
                    nc.vector.reciprocal(rno[:, s * 512:(s + 1) * 512], nrm_t[:])

        # ================= P2c: normalize q,k chan-major ====================
        with tc.tile_pool(name="c2", bufs=2) as c2p, \
             tc.tile_pool(name="c2ps", bufs=4, space="PSUM") as c2ps:
            for name, st, dst in (("q", stq_d, qT_d), ("k", stk_d, kT_d)):
                rno = rnos[name]
                for g in range(8):
                    xin = c2p.tile([128, L], f32, tag="xin")
                    nc.sync.dma_start(xin[:], st[:, g, :])
                    xo = c2p.tile([128, L], f32, tag="xo")
                    for s in range(NS):
                        ps = c2ps.tile([128, 512], f32, tag="bc")
                        nc.tensor.matmul(ps[:], e64[:, g * 128:(g + 1) * 128],
                                         rno[:, s * 512:(s + 1) * 512],
                                         start=True, stop=True)
                        nc.vector.tensor_mul(xo[:, s * 512:(s + 1) * 512],
                                             xin[:, s * 512:(s + 1) * 512], ps[:])
                    nc.sync.dma_start(dst[:, g, :], xo[:])

        # ================= P2b: transposes to tok-major =====================
        with tc.tile_pool(name="tb", bufs=3) as tbp, \
             tc.tile_pool(name="tbps", bufs=4, space="PSUM") as tbps:
            for src, dst, dt_ in ((kT_d, kf_d, f32), (stv_d, vf_d, f32),
                                  (stg_d, gatef_d, f32)):
                for t in range(NT):
                    xin = tbp.tile([128, 8, 128], f32, tag="xin")
                    nc.sync.dma_start(xin[:], src[:, :, t * 128:(t + 1) * 128])
                    xo = tbp.tile([128, 1024], dt_, tag="xo")
                    for g in range(8):
                        ps = tbps.tile([128, 128], f32, tag="tp")
                        nc.tensor.transpose(ps[:], xin[:, g, :], eye[:])
                        nc.vector.tensor_copy(xo[:, g * 128:(g + 1) * 128], ps[:])
                    nc.sync.dma_start(dst[t * 128:(t + 1) * 128, :], xo[:])

        # ================= P2R: fast-weight recurrence ======================
        win = constp.tile([128, 8, 64], f32)      # block-diag W_in  (chan, 4h+D)
        nc.sync.dma_start(win[:], wbdin0_d[:])
        woutT = constp.tile([128, 8, 64], f32)    # W_out^T block-diag
        nc.sync.dma_start(woutT[:], woutT0_d[:])
        wout = constp.tile([64, 1024], f32)       # W_out flat (4h+D, chan)
        nc.sync.dma_start(wout[:], wout0_d[:])
        maskT = constp.tile([128, 8, 64], f32)
        nc.sync.dma_start(maskT[:], maskT_d[:])
        maskF = constp.tile([64, 1024], f32)
        nc.sync.dma_start(maskF[:], maskF_d[:])
        mask16T = constp.tile([16, 256], f32)
        nc.sync.dma_start(mask16T[:], mask16_d[:])
        sel = constp.tile([64, 256], f32)
        nc.sync.dma_start(sel[:], sel_d[:])

        with tc.tile_pool(name="rin", bufs=3) as rin, \
             tc.tile_pool(name="rw", bufs=2) as rw, \
             tc.tile_pool(name="rps", bufs=2, space="PSUM") as rps, \
             tc.tile_pool(name="rpo", bufs=2, space="PSUM") as rpo, \
             tc.tile_pool(name="rpu", bufs=1, space="PSUM") as rpu:

            def softmax4(s_ps, tag):
                # s_ps (16, 64) = scores grouped [h, D]; softmax over D (4)
                nmax = rw.tile([16, 16], f32, tag=f"nm_{tag}")
                nc.vector.tensor_reduce(
                    nmax[:], s_ps[:].rearrange("p (g x) -> p g x", x=4),
                    axis=AX.X, op=ALU.max, negate=True)
                e = rw.tile([16, 64], f32, tag=f"e_{tag}")
                nc.vector.tensor_tensor(
                    e[:].rearrange("p (g x) -> p g x", x=4),
                    s_ps[:].rearrange("p (g x) -> p g x", x=4),
                    nmax[:, :, None].broadcast_to([16, 16, 4]), op=ALU.add)
                nc.scalar.activation(e[:], e[:], AF.Exp)
                gs = rw.tile([16, 16], f32, tag=f"gs_{tag}")
                nc.vector.tensor_reduce(
                    gs[:], e[:].rearrange("p (g x) -> p g x", x=4),
                    axis=AX.X, op=ALU.add)
                gr = rw.tile([16, 16], f32, tag=f"gr_{tag}")
                nc.vector.reciprocal(gr[:], gs[:])
                p = rw.tile([16, 64], f32, tag=f"p_{tag}")
                nc.vector.tensor_tensor(
                    p[:].rearrange("p (g x) -> p g x", x=4),
                    e[:].rearrange("p (g x) -> p g x", x=4),
                    gr[:, :, None].broadcast_to([16, 16, 4]), op=ALU.mult)
                return p

            def softmax16(s_ps, tag):
                # s_ps (64, 16): softmax over free dim of s/8
                nmax = rw.tile([64, 1], f32, tag=f"nm16_{tag}")
                nc.vector.tensor_reduce(nmax[:], s_ps[:], axis=AX.X,
                                        op=ALU.max, negate=True)
                nm8 = rw.tile([64, 1], f32, tag=f"nm8_{tag}")
                nc.vector.tensor_scalar_mul(nm8[:], nmax[:], 0.125)
                e = rw.tile([64, 16], f32, tag=f"e16_{tag}")
                nc.scalar.activation(e[:], s_ps[:], AF.Exp,
                                     bias=nm8[:], scale=0.125)
                rs = rw.tile([64, 1], f32, tag=f"rs_{tag}")
                nc.vector.tensor_reduce(rs[:], e[:], axis=AX.X, op=ALU.add)
                rr = rw.tile([64, 1], f32, tag=f"rr_{tag}")
                nc.vector.reciprocal(rr[:], rs[:])
                p = rw.tile([64, 16], f32, tag=f"p16_{tag}")
                nc.vector.tensor_scalar_mul(p[:], e[:], rr[:])
                return p

            def transpose_to(p_sb, P, Fr, tag):
                # (P, Fr) -> (Fr, P), Fr <= 128
                ps = rps.tile([Fr, P], f32, tag="tp")
                nc.tensor.transpose(ps[:], p_sb[:], eye[:P, :P])
                sb = rw.tile([Fr, P], f32, tag=f"tps_{tag}")
                nc.vector.tensor_copy(sb[:], ps[:])
                return sb

            with tc.For_i(0, NCK, 1) as i:
                t0 = i * CH
                KT = rin.tile([128, 8, CH], f32, tag="KT")
                nc.sync.dma_start(KT[:], kT_d[:, :, bass.ds(t0, CH)])
                QT = rin.tile([128, 8, CH], f32, tag="QT")
                nc.sync.dma_start(QT[:], qT_d[:, :, bass.ds(t0, CH)])
                VT = rin.tile([128, 8, CH], f32, tag="VT")
                nc.sync.dma_start(VT[:], stv_d[:, :, bass.ds(t0, CH)])
                Kf = rin.tile([CH, 1024], f32, tag="Kf")
                nc.sync.dma_start(Kf[:], kf_d[bass.ds(t0, CH), :])
                Vf = rin.tile([CH, 1024], f32, tag="Vf")
                nc.sync.dma_start(Vf[:], vf_d[bass.ds(t0, CH), :])
                lrc1 = rin.tile([16, CH], f32, tag="lrc1")
                nc.sync.dma_start(lrc1[:], lrT_d[0:16, bass.ds(t0, CH)])
                lrc0 = rin.tile([16, CH], f32, tag="lrc0")
                nc.sync.dma_start(lrc0[:], lrT_d[16:32, bass.ds(t0, CH)])

                # --- scores vs W_in, chunk-local attention ---
                sk_ps = rps.tile([16, 64], f32, tag="s")
                for g in range(8):
                    nc.tensor.matmul(sk_ps[:], KT[:, g, :], win[:, g, :],
                                     start=(g == 0), stop=(g == 7))
                p_k = softmax4(sk_ps, "k")
                # lr1 broadcast (16, 64): lhsT = lrT[0:16, chunk] (16,16)
                lr1_ps = rps.tile([16, 64], f32, tag="s")
                nc.tensor.matmul(lr1_ps[:], lrc1[:], e4[:],
                                 start=True, stop=True)
                k_h = rw.tile([16, 64], f32, tag="k_h")
                nc.vector.tensor_mul(k_h[:], p_k[:], lr1_ps[:])

                sq_ps = rps.tile([16, 64], f32, tag="s")
                for g in range(8):
                    nc.tensor.matmul(sq_ps[:], QT[:, g, :], win[:, g, :],
                                     start=(g == 0), stop=(g == 7))
                q_h = softmax4(sq_ps, "q")

                q_hT = transpose_to(q_h, 16, 64, "qh")
                k_hT = transpose_to(k_h, 16, 64, "kh")

                # block-diagonal expansion: q_hX = SEL * tile16(q_hT)
                q_hX = rw.tile([64, 256], f32, tag="q_hX")
                nc.vector.tensor_tensor(
                    q_hX[:].rearrange("p (h q) -> p h q", q=16),
                    sel[:].rearrange("p (h q) -> p h q", q=16),
                    q_hT[:, None, :].broadcast_to([64, 16, 16]),
                    op=ALU.mult)
                # S_catT[k', 16h+q] = sum_D k_h[k', 4h+D] q_h[q, 4h+D]
                ST_ps = rps.tile([16, 256], f32, tag="s")
                nc.tensor.matmul(ST_ps[:], k_hT[:], q_hX[:],
                                 start=True, stop=True)
                S_mT = rw.tile([16, 256], f32, tag="S_mT")
                nc.vector.tensor_mul(S_mT[:], ST_ps[:], mask16T[:])

                # o = q_h @ W_out + S_mT-applied V  (two 512-col halves)
                o_sb = rw.tile([16, 1024], f32, tag="o_sb")
                for half in range(2):
                    o_ps = rpo.tile([16, 512], f32, tag="o")
                    nc.tensor.matmul(o_ps[:], q_hT[:],
                                     wout[:, half * 512:(half + 1) * 512],
                                     start=True, stop=False)
                    for hh in range(8):
                        h = half * 8 + hh
                        nc.tensor.matmul(
                            o_ps[:, hh * 64:(hh + 1) * 64],
                            S_mT[:, 16 * h:16 * (h + 1)],
                            Vf[:, h * 64:(h + 1) * 64],
                            start=False, stop=(hh == 7))
                    nc.vector.tensor_copy(o_sb[:, half * 512:(half + 1) * 512],
                                          o_ps[:])
                nc.sync.dma_start(obuf_d[bass.ds(t0, CH), :], o_sb[:])

                # --- W_out += k_h^T @ V (flat + transposed) ---
                for half in range(2):
                    u_ps = rpu.tile([64, 512], f32, tag="u")
                    nc.tensor.matmul(u_ps[:], k_h[:],
                                     Vf[:, half * 512:(half + 1) * 512],
                                     start=True, stop=True)
                    tmp = rw.tile([64, 512], f32, tag="uf")
                    nc.vector.tensor_mul(tmp[:], u_ps[:],
                                         maskF[:, half * 512:(half + 1) * 512])
                    nc.vector.tensor_add(wout[:, half * 512:(half + 1) * 512],
                                         wout[:, half * 512:(half + 1) * 512],
                                         tmp[:])
                uT_ps = rpu.tile([128, 8, 64], f32, tag="uT")
                for g in range(8):
                    nc.tensor.matmul(uT_ps[:, g, :],
                                     Vf[:, g * 128:(g + 1) * 128], k_h[:],
                                     start=True, stop=True)
                tmpT = rw.tile([128, 8, 64], f32, tag="uTf")
                nc.vector.tensor_mul(tmpT[:], uT_ps[:], maskT[:])
                nc.vector.tensor_add(woutT[:], woutT[:], tmpT[:])

                # lr columns for this chunk
                lrin_ps = rps.tile([128, 8], f32, tag="s")
                lrout_ps = rps.tile([128, 8], f32, tag="tp")
                for g in range(8):
                    nc.tensor.matmul(lrin_ps[:, g:g + 1],
                                     e64[:, g * 128:(g + 1) * 128],
                                     lrc0[:, 0:1], start=True, stop=True)
                    nc.tensor.matmul(lrout_ps[:, g:g + 1],
                                     e64[:, g * 128:(g + 1) * 128],
                                     lrc1[:, 0:1], start=True, stop=True)
                lrin_b = rw.tile([128, 8], f32, tag="lrin_b")
                nc.vector.tensor_copy(lrin_b[:], lrin_ps[:])
                lrout_b = rw.tile([128, 8], f32, tag="lrout_b")
                nc.vector.tensor_copy(lrout_b[:], lrout_ps[:])
                lroutD_ps = rps.tile([64, 1], f32, tag="s")
                nc.tensor.matmul(lroutD_ps[:], e4[:], lrc1[:, 0:1],
                                 start=True, stop=True)
                lroutD = rw.tile([64, 1], f32, tag="lroutD")
                nc.vector.tensor_copy(lroutD[:], lroutD_ps[:])

                # --- two test-time gradient steps ---
                for it in range(2):
                    # g_out = attn(W_in, K, V): S1 = Wbd_in^T @ KT
                    S1_ps = rps.tile([64, 16], f32, tag="s")
                    for g in range(8):
                        nc.tensor.matmul(S1_ps[:], win[:, g, :], KT[:, g, :],
                                         start=(g == 0), stop=(g == 7))
                    p1 = softmax16(S1_ps, "p1")
                    p1T = transpose_to(p1, 64, 16, "p1")
                    # W_out += lroutD * maskF * (p1 @ Vf) ; halves
                    for half in range(2):
                        g1_ps = rpu.tile([64, 512], f32, tag="u")
                        nc.tensor.matmul(g1_ps[:], p1T[:],
                                         Vf[:, half * 512:(half + 1) * 512],
                                         start=True, stop=True)
                        tmp = rw.tile([64, 512], f32, tag="uf")
                        nc.vector.tensor_mul(tmp[:], g1_ps[:],
                                             maskF[:, half * 512:(half + 1) * 512])
                        nc.vector.scalar_tensor_tensor(
                            wout[:, half * 512:(half + 1) * 512], tmp[:],
                            lroutD[:],
                            wout[:, half * 512:(half + 1) * 512],
                            op0=ALU.mult, op1=ALU.add)
                    # transposed copy
                    g1T_ps = rpu.tile([128, 8, 64], f32, tag="uT")
                    for g in range(8):
                        nc.tensor.matmul(g1T_ps[:, g, :],
                                         Vf[:, g * 128:(g + 1) * 128], p1T[:],
                                         start=True, stop=True)
                    g1T = rw.tile([128, 8, 64], f32, tag="uTf")
                    nc.vector.tensor_mul(g1T[:], g1T_ps[:], maskT[:])
                    for g in range(8):
                        nc.vector.scalar_tensor_tensor(
                            woutT[:, g, :], g1T[:, g, :], lrout_b[:, g:g + 1],
                            woutT[:, g, :], op0=ALU.mult, op1=ALU.add)

                    # g_in = attn(W_out, V, K): S2 = W_outT^T @ VT
                    S2_ps = rps.tile([64, 16], f32, tag="s")
                    for g in range(8):
                        nc.tensor.matmul(S2_ps[:], woutT[:, g, :], VT[:, g, :],
                                         start=(g == 0), stop=(g == 7))
                    p2 = softmax16(S2_ps, "p2")
                    p2T = transpose_to(p2, 64, 16, "p2")
                    # W_in += lrin_b * maskT * (Kf^T-tiled @ p2T)
                    g2_ps = rpu.tile([128, 8, 64], f32, tag="uT")
                    for g in range(8):
                        nc.tensor.matmul(g2_ps[:, g, :],
                                         Kf[:, g * 128:(g + 1) * 128], p2T[:],
                                         start=True, stop=True)
                    g2 = rw.tile([128, 8, 64], f32, tag="uTf")
                    nc.vector.tensor_mul(g2[:], g2_ps[:], maskT[:])
                    for g in range(8):
                        nc.vector.scalar_tensor_tensor(
                            win[:, g, :], g2[:, g, :], lrin_b[:, g:g + 1],
                            win[:, g, :], op0=ALU.mult, op1=ALU.add)

        # ================= P3: layernorm, gate, out matmul ==================
        with tc.tile_pool(name="f3", bufs=2) as f3p, \
             tc.tile_pool(name="f3w", bufs=1) as f3w, \
             tc.tile_pool(name="f3ps", bufs=4, space="PSUM") as f3ps, \
             tc.tile_pool(name="f3po", bufs=2, space="PSUM") as f3po:
            w2 = f3w.tile([128, 16, 1024], b16)
            nc.sync.dma_start(w2[:], w2_d[:])
            for t in range(NT):
                o = f3p.tile([128, 1024], f32, tag="o")
                nc.sync.dma_start(o[:], obuf_d[t * 128:(t + 1) * 128, :])
                gf = f3p.tile([128, 1024], f32, tag="gf")
                nc.sync.dma_start(gf[:], gatef_d[t * 128:(t + 1) * 128, :])
                gT = f3p.tile([128, 8, 128], b16, tag="gT")
                nc.sync.dma_start(gT[:], gateT_d[:, :, t * 128:(t + 1) * 128])
                # LN over 64-groups
                ssum = f3p.tile([128, 16], f32, tag="ssum")
                nc.vector.tensor_reduce(
                    ssum[:], o[:].rearrange("p (g x) -> p g x", x=64),
                    axis=AX.X, op=ALU.add)
                mu = f3p.tile([128, 16], f32, tag="mu")
                nc.vector.tensor_scalar_mul(mu[:], ssum[:], 1.0 / 64)
                xm = f3p.tile([128, 1024], f32, tag="xm")
                nc.vector.tensor_tensor(
                    xm[:].rearrange("p (g x) -> p g x", x=64),
                    o[:].rearrange("p (g x) -> p g x", x=64),
                    mu[:, :, None].broadcast_to([128, 16, 64]), op=ALU.subtract)
                sq2 = f3p.tile([128, 1024], f32, tag="sq2")
                nc.vector.tensor_mul(sq2[:], xm[:], xm[:])
                var = f3p.tile([128, 16], f32, tag="var")
                nc.vector.tensor_reduce(
                    var[:], sq2[:].rearrange("p (g x) -> p g x", x=64),
                    axis=AX.X, op=ALU.add)
                sd = f3p.tile([128, 16], f32, tag="sd")
                nc.scalar.activation(sd[:], var[:], AF.Sqrt,
                                     bias=c_eps[:], scale=1.0 / 64)
                rsd = f3p.tile([128, 16], f32, tag="rsd")
                nc.vector.reciprocal(rsd[:], sd[:])
                xn = f3p.tile([128, 1024], f32, tag="xn")
                nc.vector.tensor_tensor(
                    xn[:].rearrange("p (g x) -> p g x", x=64),
                    xm[:].rearrange("p (g x) -> p g x", x=64),
                    rsd[:, :, None].broadcast_to([128, 16, 64]), op=ALU.mult)
                xg = f3p.tile([128, 1024], b16, tag="xg")
                nc.vector.tensor_mul(xg[:], xn[:], gf[:])
                # transpose xg to (chan, tok) tiles
                xgT = f3p.tile([128, 8, 128], b16, tag="xgT")
                for g in range(8):
                    ps = f3ps.tile([128, 128], b16, tag="tp")
                    nc.tensor.transpose(ps[:], xg[:, g * 128:(g + 1) * 128],
                                        eyeb[:])
                    nc.vector.tensor_copy(xgT[:, g, :], ps[:])
                oo = f3p.tile([128, 1024], b16, tag="oo")
                for half in range(2):
                    ps = f3po.tile([128, 512], f32, tag="out")
                    for kg in range(16):
                        lhsT = xgT[:, kg, :] if kg < 8 else gT[:, kg - 8, :]
                        nc.tensor.matmul(ps[:], lhsT,
                                         w2[:, kg, half * 512:(half + 1) * 512],
                                         start=(kg == 0), stop=(kg == 15))
                    nc.vector.tensor_copy(oo[:, half * 512:(half + 1) * 512],
                                          ps[:])
                nc.sync.dma_start(out_d[t * 128:(t + 1) * 128, :], oo[:])

    nc.compile()
    return nc


# ======================= host-side preparation =============================

def fold(M):
    """(1024, X) -> (128, 8, X) with chan = g*128 + p."""
    return np.ascontiguousarray(
        np.asarray(M).reshape(8, 128, -1).transpose(1, 0, 2))


def prep_weights(Wq, Wk, Wv, Wlr, Wg, Wo, cq, ck, cv, W_in_init, W_out_init,
                 ln_g, ln_b):
    """Build the shared (batch-independent) input tensors."""
    W = {}
    w4 = np.concatenate([np.asarray(x, np.float32).T
                         for x in (Wq, Wk, Wv, Wg)], axis=1)   # (1024, 4096)
    W["w4T"] = fold(w4).astype(bf16)
    perm = [2 * h + 1 for h in range(16)] + [2 * h for h in range(16)]
    W["wlrT"] = fold(np.asarray(Wlr, np.float32)[perm].T).astype(bf16)
    lng = np.tile(np.asarray(ln_g, np.float32), 16)
    lnb = np.tile(np.asarray(ln_b, np.float32), 16)
    WoT = np.asarray(Wo, np.float32).T                          # (chan, out)
    W2 = np.concatenate([lng[:, None] * WoT, lnb[:, None] * WoT], axis=0)
    W["w2"] = np.ascontiguousarray(
        W2.reshape(16, 128, 1024).transpose(1, 0, 2)).astype(bf16)
    convw = np.zeros((1024, 12), np.float32)
    for j, cw in enumerate((cq, ck, cv)):
        convw[:, 4 * j:4 * j + 4] = np.asarray(cw, np.float32)
        convw[:, 4 * j + 3] += 1.0
    W["convw"] = fold(convw)
    Win0 = np.asarray(W_in_init, np.float32)[0]    # (4, 16, 64)
    Wout0 = np.asarray(W_out_init, np.float32)[0]
    wbdin0 = np.zeros((1024, 64), np.float32)
    woutT0 = np.zeros((1024, 64), np.float32)
    maskT = np.zeros((1024, 64), np.float32)
    for h in range(16):
        for d in range(64):
            c = 64 * h + d
            wbdin0[c, 4 * h:4 * h + 4] = Win0[:, h, d]
            woutT0[c, 4 * h:4 * h + 4] = Wout0[:, h, d]
            maskT[c, 4 * h:4 * h + 4] = 1.0
    W["wbdin0"] = fold(wbdin0)
    W["woutT0"] = fold(woutT0)
    W["maskT"] = fold(maskT)
    wout0 = np.zeros((64, 1024), np.float32)
    maskF = np.zeros((64, 1024), np.float32)
    for h in range(16):
        for D in range(4):
            wout0[4 * h + D, 64 * h:64 * h + 64] = Wout0[D, h]
            maskF[4 * h + D, 64 * h:64 * h + 64] = 1.0
    W["wout0"] = wout0
    W["maskF"] = maskF
    mask16 = np.zeros((16, 256), np.float32)
    trilT = np.tril(np.ones((16, 16), np.float32)).T   # [k', q] = 1 if k' <= q
    for h in range(16):
        mask16[:, 16 * h:16 * h + 16] = trilT
    W["mask16"] = mask16
    e4 = np.zeros((16, 64), np.float32)
    e64 = np.zeros((16, 1024), np.float32)
    for h in range(16):
        e4[h, 4 * h:4 * h + 4] = 1.0
        e64[h, 64 * h:64 * h + 64] = 1.0
    W["e4"] = e4
    W["e64"] = e64
    ones2 = np.zeros((128, 2), np.float32)
    ones2[:64, 0] = 1.0
    ones2[64:, 1] = 1.0
    W["ones2"] = ones2
    onesbT = np.zeros((1024, 16), np.float32)
    for c in range(1024):
        onesbT[c, c // 64] = 1.0
    W["onesbT"] = fold(onesbT)
    sel = np.zeros((64, 16, 16), np.float32)
    for h in range(16):
        sel[4 * h:4 * h + 4, h, :] = 1.0
    W["sel"] = sel.reshape(64, 256)
    W["eye"] = np.eye(128, dtype=np.float32)
    return W


def make_in_map(hs_b, W):
    """Per-core inputs: hs_b (L, 1024) f32 for this core's batch."""
    m = dict(W)
    m["hsT"] = fold(np.asarray(hs_b, np.float32).T).astype(bf16)
    return m


# ======================= kernel entry point ================================

_NC = {}
LAST_EXEC_NS = []
B = 4
L = 4096
BPC = 1                      # batches per core -> 4 cores


def _run(nc, in_maps):
    import time
    from concourse.bass_utils import run_bass_kernel_spmd
    t0 = time.perf_counter()
    res = run_bass_kernel_spmd(nc, in_maps, core_ids=list(range(len(in_maps))))
    dt = time.perf_counter() - t0
    if res.exec_time_ns is not None:
        LAST_EXEC_NS.append(res.exec_time_ns)
    else:
        LAST_EXEC_NS.append(int(dt * 1e9))
    return res.results


def kernel(hidden_states, Wq, Wk, Wv, Wlr, Wg, Wo, cq, ck, cv,
           W_in_init, W_out_init, ln_g, ln_b):
    use_lnb = bool(np.any(np.asarray(ln_b, np.float32) != 0.0))
    key = (BPC, use_lnb)
    if key not in _NC:
        _NC[key] = build_fused(L=L, BPC=BPC, use_lnb=use_lnb)
    W = prep_weights(Wq, Wk, Wv, Wlr, Wg, Wo, cq, ck, cv,
                     W_in_init, W_out_init, ln_g, ln_b, use_lnb=use_lnb)
    hs = np.asarray(hidden_states, np.float32)
    ncores = B // BPC
    in_maps = [make_in_map([hs[c * BPC + b] for b in range(BPC)], W)
               for c in range(ncores)]
    results = _run(_NC[key], in_maps)
    out = np.concatenate([np.asarray(results[c]["out"], np.float32)
                          for c in range(ncores)])
    return out.reshape(B, L, 1024)
